# revision 5
# baseline (speedup 1.0000x reference)
"""Causal self-attention Trainium2 kernel (8-core head-parallel).

Full inputs in, full output out. Sharding strategy:
  - 16 heads / 8 cores -> 2 heads per core, both batch elems (4 (b,h) attention
    problems per core).
  - QKV projection column-parallel: each core gets w_attn[:, cols-of-its-heads]
    as a [1024, 384] slice (q 128 | k 128 | v 128), q pre-scaled by 1/sqrt(D).
  - c_proj row-parallel: each core gets w_proj[128c:128c+128, :] and produces a
    partial [B, C, T] output (transposed); host sums the 8 partials (the
    all-reduce of the row-parallel projection), transposes back, adds bias.

On-device layout (per core) keeps everything transposed to avoid transposes:
  xT [C=1024, TOK=4096] bf16 (host-pretransposed)
  qkv^T = Wslice^T @ xT  -> q^T,k^T [128(2 heads x 64), TOK], v^T [128, TOK]
  v^T is PE-transposed into V tiles [128 s, 65] with a ones column (col 64),
  so the AV matmul also produces the softmax denominator Z as row 64.
  S^T[s, q] = k^T-tile (stationary) x q^T (moving); exp on ACT with bias -4;
  causal handled by only computing q >= s-tile-start plus one [128,128]
  triangular mask multiply per diagonal tile (on GPSIMD).
  y_aug^T [65, q] accumulates over s-tiles in PSUM; normalization multiplies by
  a PE-broadcast of 1/Z; out^T[b] = wp_slice^T @ y^T done per 128-col tile.
"""

import math

import numpy as np
import ml_dtypes

import concourse.bass as bass
from concourse import bacc
import concourse.mybir as mybir
from concourse.tile import TileContext
from concourse.bass_utils import run_bass_kernel_spmd

BF16 = mybir.dt.bfloat16
F32 = mybir.dt.float32
NPBF16 = ml_dtypes.bfloat16

P = 128
B, T, C = 2, 2048, 1024
H, D = 16, 64
NCORES = 8
HPC = H // NCORES          # heads per core
TOK = B * T                # 4096 flattened tokens (b-major)
NCT = C // P               # 8 contraction tiles for the projections
NTC = TOK // 512           # 8 token chunks of 512
QW = 1024                  # q window width for attention inner loop
EXP_BIAS = -4.0            # exp(s - 4): cancels in normalization, guards tail


def build_nc(with_bias: bool) -> bacc.Bacc:
    nc = bacc.Bacc(None, target_bir_lowering=False)

    xt = nc.dram_tensor("xt", [C, TOK], BF16, kind="ExternalInput")
    wqkv = nc.dram_tensor("wqkv", [C, 3 * P], BF16, kind="ExternalInput")
    wp = nc.dram_tensor("wp", [P, C], BF16, kind="ExternalInput")
    tri = nc.dram_tensor("tri", [P, P], BF16, kind="ExternalInput")
    ident = nc.dram_tensor("ident", [P, P], BF16, kind="ExternalInput")
    ones64 = nc.dram_tensor("ones64", [1, 64], F32, kind="ExternalInput")
    if with_bias:
        bqkv = nc.dram_tensor("bqkv", [1, 3 * P], BF16, kind="ExternalInput")
        ones512 = nc.dram_tensor("ones512", [1, 512], BF16, kind="ExternalInput")
    outT = nc.dram_tensor("outT", [B, C, T], BF16, kind="ExternalOutput")

    EXP = mybir.ActivationFunctionType.Exp

    with TileContext(nc) as tc:
        with (
            tc.tile_pool(name="consts", bufs=1) as consts,
            tc.tile_pool(name="px", bufs=1) as px,
            tc.tile_pool(name="pqkv", bufs=1) as pqkv,
            tc.tile_pool(name="py", bufs=1) as py,
            tc.tile_pool(name="pwork", bufs=2) as pwork,
        ):
            # ---- constant / input loads ----
            wqkv_sb = consts.tile([P, NCT, 3 * P], BF16)
            for ct in range(NCT):
                nc.sync.dma_start(wqkv_sb[:, ct, :], wqkv[ct * P:(ct + 1) * P, :])
            wp_sb = consts.tile([P, C], BF16)
            nc.sync.dma_start(wp_sb, wp[:, :])
            tri_sb = consts.tile([P, P], BF16)
            nc.sync.dma_start(tri_sb, tri[:, :])
            ident_sb = consts.tile([P, P], BF16)
            nc.sync.dma_start(ident_sb, ident[:, :])
            ones64_sb = consts.tile([1, 64], F32)
            nc.sync.dma_start(ones64_sb, ones64[:, :])
            expb = consts.tile([P, 1], F32)
            nc.vector.memset(expb, EXP_BIAS)
            if with_bias:
                bqkv_sb = consts.tile([1, 3 * P], BF16)
                nc.sync.dma_start(bqkv_sb, bqkv[:, :])
                ones512_sb = consts.tile([1, 512], BF16)
                nc.sync.dma_start(ones512_sb, ones512[:, :])

            xt_sb = px.tile([P, NCT, TOK], BF16)
            for ct in range(NCT):
                nc.sync.dma_start(xt_sb[:, ct, :], xt[ct * P:(ct + 1) * P, :])

            # qk^T slabs: ft=0 -> q^T (rows: h0 0:64, h1 64:128), ft=1 -> k^T
            qkT = pqkv.tile([P, 2, TOK], BF16)
            vT = pqkv.tile([P, TOK], BF16)
            # V tiles [s, d] with ones column at 64; index = (b*HPC+h)*T/P + st
            V = pqkv.tile([P, B * HPC * (T // P), 65], BF16)
            nc.vector.memset(V, 1.0)  # pre-fill so col 64 is the ones column
            yT = py.tile([P, B, T], BF16)

            # ---- phase 1: QKV projection (q^T, k^T, v^T) ----
            with (
                tc.tile_pool(name="ps_qkv", bufs=5, space="PSUM") as ps_qkv,
                tc.tile_pool(name="ps_t", bufs=2, space="PSUM") as ps_t,
            ):
                for ft in range(3):
                    for tcq in range(NTC // 4):
                        psums = []
                        for q in range(4):
                            pt = ps_qkv.tile([P, 512], F32, tag="qkv", name=f"qkvps_{ft}_{tcq}_{q}")
                            psums.append(pt)
                        for ct in range(NCT):
                            for q in range(4):
                                tcc = tcq * 4 + q
                                nc.tensor.matmul(
                                    psums[q],
                                    wqkv_sb[:, ct, ft * P:(ft + 1) * P],
                                    xt_sb[:, ct, tcc * 512:(tcc + 1) * 512],
                                    start=(ct == 0),
                                    stop=(ct == NCT - 1 and not with_bias),
                                )
                        if with_bias:
                            for q in range(4):
                                nc.tensor.matmul(
                                    psums[q],
                                    bqkv_sb[0:1, ft * P:(ft + 1) * P],
                                    ones512_sb[0:1, :],
                                    start=False,
                                    stop=True,
                                )
                        for q in range(4):
                            tcc = tcq * 4 + q
                            if ft < 2:
                                nc.vector.tensor_copy(
                                    qkT[:, ft, tcc * 512:(tcc + 1) * 512], psums[q]
                                )
                            else:
                                nc.vector.tensor_copy(
                                    vT[:, tcc * 512:(tcc + 1) * 512], psums[q]
                                )

                # v^T -> V tiles (PE transpose, both heads at once)
                for b in range(B):
                    for st in range(T // P):
                        pt = ps_t.tile([P, P], BF16, tag="vt", name=f"vtps_{b}_{st}")
                        nc.tensor.transpose(
                            pt, vT[:, b * T + st * P: b * T + (st + 1) * P], ident_sb
                        )
                        for h in range(HPC):
                            vidx = (b * HPC + h) * (T // P) + st
                            nc.vector.tensor_copy(
                                V[:, vidx, 0:64], pt[:, h * 64:(h + 1) * 64]
                            )

            # ---- phase 2: attention ----
            with tc.tile_pool(name="ps_att", bufs=1, space="PSUM") as ps_att:
                for b in range(B):
                    for qh in range(T // QW):
                        qbase = qh * QW
                        n_st = (qbase + QW) // P
                        ys = []
                        for h in range(HPC):
                            y = ps_att.tile([P, QW], F32, tag="y", bufs=2,
                                            name=f"yps_{b}_{qh}_{h}")
                            ys.append(y)
                        # last s-tile contributing to each 512-wide output bank
                        last_st = {0: (qbase + 512) // P - 1, 1: n_st - 1}
                        for st in range(n_st):
                            s0 = st * P
                            qa = max(qbase, s0)          # global start of q span
                            w = qbase + QW - qa          # active width
                            for h in range(HPC):
                                ps = ps_att.tile([P, QW], F32, tag="s", bufs=2,
                                                 name=f"sps_{b}_{qh}_{st}_{h}")
                                for c0 in range(0, w, 512):
                                    cw = min(512, w - c0)
                                    nc.tensor.matmul(
                                        ps[:, c0:c0 + cw],
                                        qkT[h * 64:(h + 1) * 64, 1,
                                            b * T + s0: b * T + s0 + P],
                                        qkT[h * 64:(h + 1) * 64, 0,
                                            b * T + qa + c0: b * T + qa + c0 + cw],
                                        start=True, stop=True,
                                    )
                                es = pwork.tile([P, QW], BF16, tag="expS", bufs=4,
                                                name=f"es_{b}_{qh}_{st}_{h}")
                                nc.scalar.activation(
                                    es[:, 0:w], ps[:, 0:w], EXP, bias=expb
                                )
                                if s0 >= qbase:
                                    # diagonal tile: mask s > q within first 128 cols
                                    nc.gpsimd.tensor_mul(
                                        es[:, 0:P], es[:, 0:P], tri_sb
                                    )
                                # AV accumulation; output cols offset by qa-qbase
                                off = qa - qbase
                                vidx = (b * HPC + h) * (T // P) + st
                                for k in range(2):
                                    lo = max(off, k * 512)
                                    hi = (k + 1) * 512
                                    if lo >= hi:
                                        continue
                                    nc.tensor.matmul(
                                        ys[h][0:65, lo:hi],
                                        V[:, vidx, :],
                                        es[:, lo - off:hi - off],
                                        start=(st == 0),
                                        stop=(st == last_st[k]),
                                    )
                        # normalize: y^T[d,q] * broadcast(1/Z[q])
                        for h in range(HPC):
                            r = pwork.tile([1, QW], F32, tag="r", bufs=2,
                                           name=f"r_{b}_{qh}_{h}")
                            nc.vector.reciprocal(r, ys[h][64:65, :])
                            rb = ps_att.tile([P, QW], F32, tag="s", bufs=2,
                                             name=f"rb_{b}_{qh}_{h}")
                            for c0 in range(0, QW, 512):
                                nc.tensor.matmul(
                                    rb[0:64, c0:c0 + 512],
                                    ones64_sb,
                                    r[:, c0:c0 + 512],
                                    start=True, stop=True,
                                )
                            ynum = pwork.tile([64, QW], BF16, tag="ynum", bufs=2,
                                              name=f"ynum_{b}_{qh}_{h}")
                            nc.scalar.copy(ynum, ys[h][0:64, :])
                            nc.vector.tensor_mul(
                                yT[h * 64:(h + 1) * 64, b, qbase:qbase + QW],
                                ynum,
                                rb[0:64, :],
                            )

            # ---- phase 3: output projection (partial, transposed) ----
            with tc.tile_pool(name="ps_o", bufs=6, space="PSUM") as ps_o:
                for of in range(NCT):
                    for b in range(B):
                        for tcc in range(T // 512):
                            po = ps_o.tile([P, 512], F32, tag="o",
                                           name=f"ops_{of}_{b}_{tcc}")
                            nc.tensor.matmul(
                                po,
                                wp_sb[:, of * P:(of + 1) * P],
                                yT[:, b, tcc * 512:(tcc + 1) * 512],
                                start=True, stop=True,
                            )
                            ot = pwork.tile([P, 512], BF16, tag="ot", bufs=4,
                                            name=f"ot_{of}_{b}_{tcc}")
                            nc.vector.tensor_copy(ot, po)
                            nc.sync.dma_start(
                                outT[b, of * P:(of + 1) * P,
                                     tcc * 512:(tcc + 1) * 512],
                                ot,
                            )
    nc.compile()
    return nc


_CACHE = {}


def _get_nc(with_bias: bool) -> bacc.Bacc:
    if with_bias not in _CACHE:
        _CACHE[with_bias] = build_nc(with_bias)
    return _CACHE[with_bias]


def _prep_inputs(x, w_attn, b_attn, w_proj):
    """Host-side shard + layout prep. Returns per-core in_maps."""
    xf = np.ascontiguousarray(
        np.asarray(x, dtype=np.float32).reshape(TOK, C).T
    ).astype(NPBF16)                                   # x^T [C, TOK]
    w = np.asarray(w_attn, dtype=np.float32)
    ba = np.asarray(b_attn, dtype=np.float32)
    wpj = np.asarray(w_proj, dtype=np.float32)
    scale = 1.0 / math.sqrt(D)
    with_bias = bool(np.any(ba))

    tri_np = np.triu(np.ones((P, P), dtype=np.float32)).astype(NPBF16)
    id_np = np.eye(P, dtype=np.float32).astype(NPBF16)
    ones64_np = np.ones((1, 64), dtype=np.float32)
    ones512_np = np.ones((1, 512), dtype=np.float32).astype(NPBF16)

    in_maps = []
    for c in range(NCORES):
        lo, hi = c * HPC * D, (c + 1) * HPC * D        # 128-wide head slice
        wq = w[:, lo:hi] * scale
        wk = w[:, C + lo:C + hi]
        wv = w[:, 2 * C + lo:2 * C + hi]
        wqkv_c = np.concatenate([wq, wk, wv], axis=1).astype(NPBF16)
        wp_c = np.ascontiguousarray(wpj[lo:hi, :]).astype(NPBF16)
        m = {
            "xt": xf,
            "wqkv": wqkv_c,
            "wp": wp_c,
            "tri": tri_np,
            "ident": id_np,
            "ones64": ones64_np,
        }
        if with_bias:
            bq = ba[lo:hi] * scale
            bk = ba[C + lo:C + hi]
            bv = ba[2 * C + lo:2 * C + hi]
            m["bqkv"] = np.concatenate([bq, bk, bv])[None, :].astype(NPBF16)
            m["ones512"] = ones512_np
        in_maps.append(m)
    return in_maps, with_bias


def _combine(results, b_proj):
    acc = np.zeros((B, C, T), dtype=np.float32)
    for r in results:
        acc += np.asarray(r["outT"], dtype=np.float32)
    out = np.transpose(acc, (0, 2, 1))                 # [B, T, C]
    out = out + np.asarray(b_proj, dtype=np.float32)[None, None, :]
    return np.ascontiguousarray(out.astype(np.float32))


def run(x, w_attn, b_attn, w_proj, b_proj, trace=False, trace_cores=None):
    in_maps, with_bias = _prep_inputs(x, w_attn, b_attn, w_proj)
    nc = _get_nc(with_bias)
    res = run_bass_kernel_spmd(
        nc, in_maps, core_ids=list(range(NCORES)),
        trace=trace, trace_cores=trace_cores,
    )
    return _combine(res.results, b_proj), res


def kernel(x, w_attn, b_attn, w_proj, b_proj):
    out, _ = run(x, w_attn, b_attn, w_proj, b_proj, trace=False)
    return out


# revision 7
# speedup vs baseline: 1.0626x; 1.0626x over previous
"""Causal self-attention Trainium2 kernel (8-core head-parallel).

Full inputs in, full output out. Sharding strategy:
  - 16 heads / 8 cores -> 2 heads per core, both batch elems (4 (b,h) attention
    problems per core).
  - QKV projection column-parallel: each core gets w_attn[:, cols-of-its-heads]
    as a [1024, 384] slice (q 128 | k 128 | v 128), q pre-scaled by 1/sqrt(D).
  - c_proj row-parallel: each core gets w_proj[128c:128c+128, :] and produces a
    partial [B, C, T] output (transposed); host sums the 8 partials (the
    all-reduce of the row-parallel projection), transposes back, adds bias.

On-device layout (per core) keeps everything transposed to avoid transposes:
  xT [C=1024, TOK=4096] bf16 (host-pretransposed)
  qkv^T = Wslice^T @ xT  -> q^T,k^T [128(2 heads x 64), TOK], v^T [128, TOK]
  v^T is PE-transposed into V tiles [128 s, 65] with a ones column (col 64),
  so the AV matmul also produces the softmax denominator Z as row 64.
  S^T[s, q] = k^T-tile (stationary) x q^T (moving); exp on ACT with bias -4;
  causal handled by only computing q >= s-tile-start plus one [128,128]
  triangular mask multiply per diagonal tile (on GPSIMD).
  y_aug^T [65, q] accumulates over s-tiles in PSUM; normalization multiplies by
  a PE-broadcast of 1/Z; out^T[b] = wp_slice^T @ y^T done per 128-col tile.
"""

import math

import numpy as np
import ml_dtypes

import concourse.bass as bass
from concourse import bacc
import concourse.mybir as mybir
from concourse.tile import TileContext
from concourse.bass_utils import run_bass_kernel_spmd

BF16 = mybir.dt.bfloat16
F32 = mybir.dt.float32
NPBF16 = ml_dtypes.bfloat16

P = 128
B, T, C = 2, 2048, 1024
H, D = 16, 64
NCORES = 8
HPC = H // NCORES          # heads per core
TOK = B * T                # 4096 flattened tokens (b-major)
NCT = C // P               # 8 contraction tiles for the projections
NTC = TOK // 512           # 8 token chunks of 512
QW = 1024                  # q window width for attention inner loop
EXP_BIAS = -4.0            # exp(s - 4): cancels in normalization, guards tail


def build_nc(with_bias: bool) -> bacc.Bacc:
    nc = bacc.Bacc(None, target_bir_lowering=False)

    xt = nc.dram_tensor("xt", [C, TOK], BF16, kind="ExternalInput")
    wqkv = nc.dram_tensor("wqkv", [C, 3 * P], BF16, kind="ExternalInput")
    wp = nc.dram_tensor("wp", [P, C], BF16, kind="ExternalInput")
    tri = nc.dram_tensor("tri", [P, P], BF16, kind="ExternalInput")
    ident = nc.dram_tensor("ident", [P, P], BF16, kind="ExternalInput")
    ones64 = nc.dram_tensor("ones64", [1, 64], BF16, kind="ExternalInput")
    if with_bias:
        bqkv = nc.dram_tensor("bqkv", [1, 3 * P], BF16, kind="ExternalInput")
        ones512 = nc.dram_tensor("ones512", [1, 512], BF16, kind="ExternalInput")
    outT = nc.dram_tensor("outT", [B, C, T], BF16, kind="ExternalOutput")

    EXP = mybir.ActivationFunctionType.Exp
    LOG = mybir.ActivationFunctionType.Ln

    with TileContext(nc) as tc:
        with (
            tc.tile_pool(name="consts", bufs=1) as consts,
            tc.tile_pool(name="px", bufs=1) as px,
            tc.tile_pool(name="pqkv", bufs=1) as pqkv,
            tc.tile_pool(name="py", bufs=1) as py,
            tc.tile_pool(name="pwork", bufs=2) as pwork,
        ):
            # ---- constant / input loads ----
            wqkv_sb = consts.tile([P, NCT, 3 * P], BF16)
            for ct in range(NCT):
                nc.sync.dma_start(wqkv_sb[:, ct, :], wqkv[ct * P:(ct + 1) * P, :])
            wp_sb = consts.tile([P, C], BF16)
            nc.sync.dma_start(wp_sb, wp[:, :])
            tri_sb = consts.tile([P, P], BF16)
            nc.sync.dma_start(tri_sb, tri[:, :])
            ident_sb = consts.tile([P, P], BF16)
            nc.sync.dma_start(ident_sb, ident[:, :])
            ones64_sb = consts.tile([1, 64], BF16)
            nc.sync.dma_start(ones64_sb, ones64[:, :])
            expb = consts.tile([P, 1], F32)
            nc.vector.memset(expb, EXP_BIAS)
            if with_bias:
                bqkv_sb = consts.tile([1, 3 * P], BF16)
                nc.sync.dma_start(bqkv_sb, bqkv[:, :])
                ones512_sb = consts.tile([1, 512], BF16)
                nc.sync.dma_start(ones512_sb, ones512[:, :])

            xt_sb = px.tile([P, NCT, TOK], BF16)
            for ct in range(NCT):
                nc.sync.dma_start(xt_sb[:, ct, :], xt[ct * P:(ct + 1) * P, :])

            # qk^T slabs: ft=0 -> q^T (rows: h0 0:64, h1 64:128), ft=1 -> k^T
            qkT = pqkv.tile([P, 2, TOK], BF16)
            vT = pqkv.tile([P, TOK], BF16)
            # V tiles [s, d] with ones column at 64; index = (b*HPC+h)*T/P + st
            V = pqkv.tile([P, B * HPC * (T // P), 65], BF16)
            nc.vector.memset(V, 1.0)  # pre-fill so col 64 is the ones column
            yT = py.tile([P, B, T], BF16)

            # ---- phase 1: QKV projection (q^T, k^T, v^T) ----
            with (
                tc.tile_pool(name="ps_qkv", bufs=5, space="PSUM") as ps_qkv,
                tc.tile_pool(name="ps_t", bufs=2, space="PSUM") as ps_t,
            ):
                for ft in range(3):
                    for tcq in range(NTC // 4):
                        psums = []
                        for q in range(4):
                            pt = ps_qkv.tile([P, 512], F32, tag="qkv", name=f"qkvps_{ft}_{tcq}_{q}")
                            psums.append(pt)
                        for ct in range(NCT):
                            for q in range(4):
                                tcc = tcq * 4 + q
                                nc.tensor.matmul(
                                    psums[q],
                                    wqkv_sb[:, ct, ft * P:(ft + 1) * P],
                                    xt_sb[:, ct, tcc * 512:(tcc + 1) * 512],
                                    start=(ct == 0),
                                    stop=(ct == NCT - 1 and not with_bias),
                                )
                        if with_bias:
                            for q in range(4):
                                nc.tensor.matmul(
                                    psums[q],
                                    bqkv_sb[0:1, ft * P:(ft + 1) * P],
                                    ones512_sb[0:1, :],
                                    start=False,
                                    stop=True,
                                )
                        for q in range(4):
                            tcc = tcq * 4 + q
                            if ft < 2:
                                nc.scalar.copy(
                                    qkT[:, ft, tcc * 512:(tcc + 1) * 512], psums[q]
                                )
                            else:
                                nc.scalar.copy(
                                    vT[:, tcc * 512:(tcc + 1) * 512], psums[q]
                                )

                # v^T -> V tiles (PE transpose, both heads at once)
                for b in range(B):
                    for st in range(T // P):
                        pt = ps_t.tile([P, P], BF16, tag="vt", name=f"vtps_{b}_{st}")
                        nc.tensor.transpose(
                            pt, vT[:, b * T + st * P: b * T + (st + 1) * P], ident_sb
                        )
                        for h in range(HPC):
                            vidx = (b * HPC + h) * (T // P) + st
                            nc.vector.tensor_copy(
                                V[:, vidx, 0:64], pt[:, h * 64:(h + 1) * 64]
                            )

            # ---- phase 2: attention ----
            with tc.tile_pool(name="ps_att", bufs=1, space="PSUM") as ps_att:
                for b in range(B):
                    for qh in range(T // QW):
                        qbase = qh * QW
                        n_st = (qbase + QW) // P
                        ys = []
                        for h in range(HPC):
                            y = ps_att.tile([P, QW], F32, tag="att", bufs=4,
                                            name=f"yps_{b}_{qh}_{h}")
                            ys.append(y)
                        # last s-tile contributing to each 512-wide output bank
                        last_st = {0: (qbase + 512) // P - 1, 1: n_st - 1}
                        for st in range(n_st):
                            s0 = st * P
                            qa = max(qbase, s0)          # global start of q span
                            w = qbase + QW - qa          # active width
                            for h in range(HPC):
                                ps = ps_att.tile([P, QW], F32, tag="att", bufs=4,
                                                 name=f"sps_{b}_{qh}_{st}_{h}")
                                for c0 in range(0, w, 512):
                                    cw = min(512, w - c0)
                                    nc.tensor.matmul(
                                        ps[:, c0:c0 + cw],
                                        qkT[h * 64:(h + 1) * 64, 1,
                                            b * T + s0: b * T + s0 + P],
                                        qkT[h * 64:(h + 1) * 64, 0,
                                            b * T + qa + c0: b * T + qa + c0 + cw],
                                        start=True, stop=True,
                                    )
                                es = pwork.tile([P, QW], BF16, tag="expS", bufs=4,
                                                name=f"es_{b}_{qh}_{st}_{h}")
                                nc.scalar.activation(
                                    es[:, 0:w], ps[:, 0:w], EXP, bias=expb
                                )
                                if s0 >= qbase:
                                    # diagonal tile: mask s > q within first 128 cols
                                    nc.gpsimd.tensor_mul(
                                        es[:, 0:P], es[:, 0:P], tri_sb
                                    )
                                # AV accumulation; output cols offset by qa-qbase
                                off = qa - qbase
                                vidx = (b * HPC + h) * (T // P) + st
                                for k in range(2):
                                    lo = max(off, k * 512)
                                    hi = (k + 1) * 512
                                    if lo >= hi:
                                        continue
                                    nc.tensor.matmul(
                                        ys[h][0:65, lo:hi],
                                        V[:, vidx, :],
                                        es[:, lo - off:hi - off],
                                        start=(st == 0),
                                        stop=(st == last_st[k]),
                                    )
                        # normalize: y^T[d,q] * broadcast(1/Z[q])
                        for h in range(HPC):
                            lz = pwork.tile([1, QW], F32, tag="lz", bufs=2,
                                            name=f"lz_{b}_{qh}_{h}")
                            nc.scalar.activation(lz, ys[h][64:65, :], LOG)
                            r = pwork.tile([1, QW], BF16, tag="r", bufs=2,
                                           name=f"r_{b}_{qh}_{h}")
                            nc.scalar.activation(r, lz, EXP, scale=-1.0)
                            rb = ps_att.tile([P, QW], F32, tag="att", bufs=4,
                                             name=f"rb_{b}_{qh}_{h}")
                            for c0 in range(0, QW, 512):
                                nc.tensor.matmul(
                                    rb[0:64, c0:c0 + 512],
                                    ones64_sb,
                                    r[:, c0:c0 + 512],
                                    start=True, stop=True,
                                )
                            ynum = pwork.tile([64, QW], BF16, tag="ynum", bufs=2,
                                              name=f"ynum_{b}_{qh}_{h}")
                            nc.scalar.copy(ynum, ys[h][0:64, :])
                            nc.vector.tensor_mul(
                                yT[h * 64:(h + 1) * 64, b, qbase:qbase + QW],
                                ynum,
                                rb[0:64, :],
                            )

            # ---- phase 3: output projection (partial, transposed) ----
            with tc.tile_pool(name="ps_o", bufs=6, space="PSUM") as ps_o:
                for of in range(NCT):
                    for b in range(B):
                        for tcc in range(T // 512):
                            po = ps_o.tile([P, 512], F32, tag="o",
                                           name=f"ops_{of}_{b}_{tcc}")
                            nc.tensor.matmul(
                                po,
                                wp_sb[:, of * P:(of + 1) * P],
                                yT[:, b, tcc * 512:(tcc + 1) * 512],
                                start=True, stop=True,
                            )
                            ot = pwork.tile([P, 512], BF16, tag="ot", bufs=4,
                                            name=f"ot_{of}_{b}_{tcc}")
                            if (of + tcc) % 2 == 0:
                                nc.vector.tensor_copy(ot, po)
                            else:
                                nc.scalar.copy(ot, po)
                            nc.sync.dma_start(
                                outT[b, of * P:(of + 1) * P,
                                     tcc * 512:(tcc + 1) * 512],
                                ot,
                            )
    nc.compile()
    return nc


_CACHE = {}


def _get_nc(with_bias: bool) -> bacc.Bacc:
    if with_bias not in _CACHE:
        _CACHE[with_bias] = build_nc(with_bias)
    return _CACHE[with_bias]


def _prep_inputs(x, w_attn, b_attn, w_proj):
    """Host-side shard + layout prep. Returns per-core in_maps."""
    xf = np.ascontiguousarray(
        np.asarray(x, dtype=np.float32).reshape(TOK, C).T
    ).astype(NPBF16)                                   # x^T [C, TOK]
    w = np.asarray(w_attn, dtype=np.float32)
    ba = np.asarray(b_attn, dtype=np.float32)
    wpj = np.asarray(w_proj, dtype=np.float32)
    scale = 1.0 / math.sqrt(D)
    with_bias = bool(np.any(ba))

    tri_np = np.triu(np.ones((P, P), dtype=np.float32)).astype(NPBF16)
    id_np = np.eye(P, dtype=np.float32).astype(NPBF16)
    ones64_np = np.ones((1, 64), dtype=np.float32).astype(NPBF16)
    ones512_np = np.ones((1, 512), dtype=np.float32).astype(NPBF16)

    in_maps = []
    for c in range(NCORES):
        lo, hi = c * HPC * D, (c + 1) * HPC * D        # 128-wide head slice
        wq = w[:, lo:hi] * scale
        wk = w[:, C + lo:C + hi]
        wv = w[:, 2 * C + lo:2 * C + hi]
        wqkv_c = np.concatenate([wq, wk, wv], axis=1).astype(NPBF16)
        wp_c = np.ascontiguousarray(wpj[lo:hi, :]).astype(NPBF16)
        m = {
            "xt": xf,
            "wqkv": wqkv_c,
            "wp": wp_c,
            "tri": tri_np,
            "ident": id_np,
            "ones64": ones64_np,
        }
        if with_bias:
            bq = ba[lo:hi] * scale
            bk = ba[C + lo:C + hi]
            bv = ba[2 * C + lo:2 * C + hi]
            m["bqkv"] = np.concatenate([bq, bk, bv])[None, :].astype(NPBF16)
            m["ones512"] = ones512_np
        in_maps.append(m)
    return in_maps, with_bias


def _combine(results, b_proj):
    acc = np.zeros((B, C, T), dtype=np.float32)
    for r in results:
        acc += np.asarray(r["outT"], dtype=np.float32)
    out = np.transpose(acc, (0, 2, 1))                 # [B, T, C]
    out = out + np.asarray(b_proj, dtype=np.float32)[None, None, :]
    return np.ascontiguousarray(out.astype(np.float32))


def run(x, w_attn, b_attn, w_proj, b_proj, trace=False, trace_cores=None):
    in_maps, with_bias = _prep_inputs(x, w_attn, b_attn, w_proj)
    nc = _get_nc(with_bias)
    res = run_bass_kernel_spmd(
        nc, in_maps, core_ids=list(range(NCORES)),
        trace=trace, trace_cores=trace_cores,
    )
    return _combine(res.results, b_proj), res


def kernel(x, w_attn, b_attn, w_proj, b_proj):
    out, _ = run(x, w_attn, b_attn, w_proj, b_proj, trace=False)
    return out


# revision 8
# speedup vs baseline: 1.1529x; 1.0850x over previous
"""Causal self-attention Trainium2 kernel (8-core head-parallel).

Full inputs in, full output out. Sharding strategy:
  - 16 heads / 8 cores -> 2 heads per core, both batch elems (4 (b,h) attention
    problems per core).
  - QKV projection column-parallel: each core gets w_attn[:, cols-of-its-heads]
    as a [1024, 384] slice (q 128 | k 128 | v 128), q pre-scaled by 1/sqrt(D).
  - c_proj row-parallel: each core gets w_proj[128c:128c+128, :] and produces a
    partial [B, C, T] output (transposed); host sums the 8 partials (the
    all-reduce of the row-parallel projection), transposes back, adds bias.

On-device layout (per core) keeps everything transposed to avoid transposes:
  xT [C=1024, TOK=4096] bf16 (host-pretransposed)
  qkv^T = Wslice^T @ xT  -> q^T,k^T [128(2 heads x 64), TOK], v^T [128, TOK]
  v^T is PE-transposed into V tiles [128 s, 65] with a ones column (col 64),
  so the AV matmul also produces the softmax denominator Z as row 64.
  S^T[s, q] = k^T-tile (stationary) x q^T (moving); exp on ACT with bias -4;
  causal handled by only computing q >= s-tile-start plus one [128,128]
  triangular mask multiply per diagonal tile (on GPSIMD).
  y_aug^T [65, q] accumulates over s-tiles in PSUM; normalization multiplies by
  a PE-broadcast of 1/Z; out^T[b] = wp_slice^T @ y^T done per 128-col tile.
"""

import math

import numpy as np
import ml_dtypes

import concourse.bass as bass
from concourse import bacc
import concourse.mybir as mybir
from concourse.tile import TileContext
from concourse.bass_utils import run_bass_kernel_spmd

BF16 = mybir.dt.bfloat16
F32 = mybir.dt.float32
NPBF16 = ml_dtypes.bfloat16

P = 128
B, T, C = 2, 2048, 1024
H, D = 16, 64
NCORES = 8
HPC = H // NCORES          # heads per core
TOK = B * T                # 4096 flattened tokens (b-major)
NCT = C // P               # 8 contraction tiles for the projections
NTC = TOK // 512           # 8 token chunks of 512
QW = 1024                  # q window width for attention inner loop
EXP_BIAS = -4.0            # exp(s - 4): cancels in normalization, guards tail


def build_nc(with_bias: bool) -> bacc.Bacc:
    nc = bacc.Bacc(None, target_bir_lowering=False)

    xt = nc.dram_tensor("xt", [C, TOK], BF16, kind="ExternalInput")
    wqkv = nc.dram_tensor("wqkv", [C, 3 * P], BF16, kind="ExternalInput")
    wp = nc.dram_tensor("wp", [P, C], BF16, kind="ExternalInput")
    tri = nc.dram_tensor("tri", [P, P], BF16, kind="ExternalInput")
    ident = nc.dram_tensor("ident", [P, P], BF16, kind="ExternalInput")
    ones64 = nc.dram_tensor("ones64", [1, 64], BF16, kind="ExternalInput")
    if with_bias:
        bqkv = nc.dram_tensor("bqkv", [1, 3 * P], BF16, kind="ExternalInput")
        ones512 = nc.dram_tensor("ones512", [1, 512], BF16, kind="ExternalInput")
    outT = nc.dram_tensor("outT", [B, C, T], BF16, kind="ExternalOutput")

    EXP = mybir.ActivationFunctionType.Exp
    LOG = mybir.ActivationFunctionType.Ln

    with TileContext(nc) as tc:
        with (
            tc.tile_pool(name="consts", bufs=1) as consts,
            tc.tile_pool(name="px", bufs=1) as px,
            tc.tile_pool(name="pqkv", bufs=1) as pqkv,
            tc.tile_pool(name="py", bufs=1) as py,
            tc.tile_pool(name="pwork", bufs=2) as pwork,
        ):
            # ---- constant / input loads ----
            wqkv_sb = consts.tile([P, NCT, 3 * P], BF16)
            for ct in range(NCT):
                nc.sync.dma_start(wqkv_sb[:, ct, :], wqkv[ct * P:(ct + 1) * P, :])
            wp_sb = consts.tile([P, C], BF16)
            nc.sync.dma_start(wp_sb, wp[:, :])
            tri_sb = consts.tile([P, P], BF16)
            nc.sync.dma_start(tri_sb, tri[:, :])
            ident_sb = consts.tile([P, P], BF16)
            nc.sync.dma_start(ident_sb, ident[:, :])
            ones64_sb = consts.tile([1, 64], BF16)
            nc.sync.dma_start(ones64_sb, ones64[:, :])
            expb = consts.tile([P, 1], F32)
            nc.vector.memset(expb, EXP_BIAS)
            if with_bias:
                bqkv_sb = consts.tile([1, 3 * P], BF16)
                nc.sync.dma_start(bqkv_sb, bqkv[:, :])
                ones512_sb = consts.tile([1, 512], BF16)
                nc.sync.dma_start(ones512_sb, ones512[:, :])

            xt_sb = px.tile([P, NCT, TOK], BF16)
            for ct in range(NCT):
                nc.sync.dma_start(xt_sb[:, ct, :], xt[ct * P:(ct + 1) * P, :])

            # qk^T slabs: ft=0 -> q^T (rows: h0 0:64, h1 64:128), ft=1 -> k^T
            qkT = pqkv.tile([P, 2, TOK], BF16)
            vT = pqkv.tile([P, TOK], BF16)
            # V tiles [s, d] with ones column at 64; index = (b*HPC+h)*T/P + st
            V = pqkv.tile([P, B * HPC * (T // P), 65], BF16)
            nc.vector.memset(V, 1.0)  # pre-fill so col 64 is the ones column
            yT = py.tile([P, B, T], BF16)

            # ---- phase 1: QKV projection (q^T, k^T, v^T) ----
            with (
                tc.tile_pool(name="ps_qkv", bufs=5, space="PSUM") as ps_qkv,
                tc.tile_pool(name="ps_t", bufs=2, space="PSUM") as ps_t,
            ):
                for ft in range(3):
                    for tcq in range(NTC // 4):
                        psums = []
                        for q in range(4):
                            pt = ps_qkv.tile([P, 512], F32, tag="qkv", name=f"qkvps_{ft}_{tcq}_{q}")
                            psums.append(pt)
                        for ct in range(NCT):
                            for q in range(4):
                                tcc = tcq * 4 + q
                                nc.tensor.matmul(
                                    psums[q],
                                    wqkv_sb[:, ct, ft * P:(ft + 1) * P],
                                    xt_sb[:, ct, tcc * 512:(tcc + 1) * 512],
                                    start=(ct == 0),
                                    stop=(ct == NCT - 1 and not with_bias),
                                )
                        if with_bias:
                            for q in range(4):
                                nc.tensor.matmul(
                                    psums[q],
                                    bqkv_sb[0:1, ft * P:(ft + 1) * P],
                                    ones512_sb[0:1, :],
                                    start=False,
                                    stop=True,
                                )
                        for q in range(4):
                            tcc = tcq * 4 + q
                            if ft < 2:
                                nc.scalar.copy(
                                    qkT[:, ft, tcc * 512:(tcc + 1) * 512], psums[q]
                                )
                            else:
                                nc.scalar.copy(
                                    vT[:, tcc * 512:(tcc + 1) * 512], psums[q]
                                )

                # v^T -> V tiles (PE transpose, both heads at once)
                for b in range(B):
                    for st in range(T // P):
                        pt = ps_t.tile([P, P], BF16, tag="vt", name=f"vtps_{b}_{st}")
                        nc.tensor.transpose(
                            pt, vT[:, b * T + st * P: b * T + (st + 1) * P], ident_sb
                        )
                        for h in range(HPC):
                            vidx = (b * HPC + h) * (T // P) + st
                            nc.vector.tensor_copy(
                                V[:, vidx, 0:64], pt[:, h * 64:(h + 1) * 64]
                            )

            # ---- phase 2+3: attention with interleaved normalize + proj ----
            # Normalize of window i is emitted inside window i+1's st loop, and
            # projection units are sprinkled into later windows, so the PE
            # never idles long enough for HAM to re-throttle.
            with tc.tile_pool(name="ps_att", bufs=1, space="PSUM") as ps_att:
                proj_ready = []

                def emit_proj_unit(b, of, tcc):
                    po = ps_att.tile([P, QW], F32, tag="att", bufs=4,
                                     name=f"ops_{b}_{of}_{tcc}")
                    nc.tensor.matmul(
                        po[:, 0:512],
                        wp_sb[:, of * P:(of + 1) * P],
                        yT[:, b, tcc * 512:(tcc + 1) * 512],
                        start=True, stop=True,
                    )
                    ot = pwork.tile([P, 512], BF16, tag="ot", bufs=6,
                                    name=f"ot_{b}_{of}_{tcc}")
                    if (of + tcc) % 2 == 0:
                        nc.vector.tensor_copy(ot, po[:, 0:512])
                    else:
                        nc.scalar.copy(ot, po[:, 0:512])
                    nc.sync.dma_start(
                        outT[b, of * P:(of + 1) * P,
                             tcc * 512:(tcc + 1) * 512],
                        ot,
                    )

                def sprinkle_proj(n):
                    for _ in range(min(n, len(proj_ready))):
                        emit_proj_unit(*proj_ready.pop(0))

                def emit_normalize(b, qh, ys):
                    qbase = qh * QW
                    for h in range(HPC):
                        lz = pwork.tile([1, QW], F32, tag="lz", bufs=2,
                                        name=f"lz_{b}_{qh}_{h}")
                        nc.scalar.activation(lz, ys[h][64:65, :], LOG)
                        r = pwork.tile([1, QW], BF16, tag="r", bufs=2,
                                       name=f"r_{b}_{qh}_{h}")
                        nc.scalar.activation(r, lz, EXP, scale=-1.0)
                        rb = ps_att.tile([P, QW], F32, tag="att", bufs=4,
                                         name=f"rb_{b}_{qh}_{h}")
                        for c0 in range(0, QW, 512):
                            nc.tensor.matmul(
                                rb[0:64, c0:c0 + 512],
                                ones64_sb,
                                r[:, c0:c0 + 512],
                                start=True, stop=True,
                            )
                        ynum = pwork.tile([64, QW], BF16, tag="ynum", bufs=2,
                                          name=f"ynum_{b}_{qh}_{h}")
                        nc.scalar.copy(ynum, ys[h][0:64, :])
                        nc.vector.tensor_mul(
                            yT[h * 64:(h + 1) * 64, b, qbase:qbase + QW],
                            ynum,
                            rb[0:64, :],
                        )
                    # this (b, qh) q-range of yT is final -> its proj can go
                    proj_ready.extend(
                        (b, of, 2 * qh + j)
                        for j in range(2) for of in range(NCT)
                    )

                pending = None
                for b in range(B):
                    for qh in range(T // QW):
                        qbase = qh * QW
                        n_st = (qbase + QW) // P
                        ys = []
                        for h in range(HPC):
                            y = ps_att.tile([P, QW], F32, tag="att", bufs=4,
                                            name=f"yps_{b}_{qh}_{h}")
                            ys.append(y)
                        # last s-tile contributing to each 512-wide output bank
                        last_st = {0: (qbase + 512) // P - 1, 1: n_st - 1}
                        for st in range(n_st):
                            s0 = st * P
                            qa = max(qbase, s0)          # global start of q span
                            w = qbase + QW - qa          # active width
                            for h in range(HPC):
                                ps = ps_att.tile([P, QW], F32, tag="att", bufs=4,
                                                 name=f"sps_{b}_{qh}_{st}_{h}")
                                for c0 in range(0, w, 512):
                                    cw = min(512, w - c0)
                                    nc.tensor.matmul(
                                        ps[:, c0:c0 + cw],
                                        qkT[h * 64:(h + 1) * 64, 1,
                                            b * T + s0: b * T + s0 + P],
                                        qkT[h * 64:(h + 1) * 64, 0,
                                            b * T + qa + c0: b * T + qa + c0 + cw],
                                        start=True, stop=True,
                                    )
                                es = pwork.tile([P, QW], BF16, tag="expS", bufs=8,
                                                name=f"es_{b}_{qh}_{st}_{h}")
                                nc.scalar.activation(
                                    es[:, 0:w], ps[:, 0:w], EXP, bias=expb
                                )
                                if s0 >= qbase:
                                    # diagonal tile: mask s > q within first 128 cols
                                    nc.gpsimd.tensor_mul(
                                        es[:, 0:P], es[:, 0:P], tri_sb
                                    )
                                # AV accumulation; output cols offset by qa-qbase
                                off = qa - qbase
                                vidx = (b * HPC + h) * (T // P) + st
                                for k in range(2):
                                    lo = max(off, k * 512)
                                    hi = (k + 1) * 512
                                    if lo >= hi:
                                        continue
                                    nc.tensor.matmul(
                                        ys[h][0:65, lo:hi],
                                        V[:, vidx, :],
                                        es[:, lo - off:hi - off],
                                        start=(st == 0),
                                        stop=(st == last_st[k]),
                                    )
                            if st == 2 and pending is not None:
                                emit_normalize(*pending)
                                pending = None
                            if st >= 4:
                                sprinkle_proj(2)
                        pending = (b, qh, ys)
                # tail: last window's normalize + remaining proj units
                emit_normalize(*pending)
                sprinkle_proj(len(proj_ready))
    nc.compile()
    return nc


_CACHE = {}


def _get_nc(with_bias: bool) -> bacc.Bacc:
    if with_bias not in _CACHE:
        _CACHE[with_bias] = build_nc(with_bias)
    return _CACHE[with_bias]


def _prep_inputs(x, w_attn, b_attn, w_proj):
    """Host-side shard + layout prep. Returns per-core in_maps."""
    xf = np.ascontiguousarray(
        np.asarray(x, dtype=np.float32).reshape(TOK, C).T
    ).astype(NPBF16)                                   # x^T [C, TOK]
    w = np.asarray(w_attn, dtype=np.float32)
    ba = np.asarray(b_attn, dtype=np.float32)
    wpj = np.asarray(w_proj, dtype=np.float32)
    scale = 1.0 / math.sqrt(D)
    with_bias = bool(np.any(ba))

    tri_np = np.triu(np.ones((P, P), dtype=np.float32)).astype(NPBF16)
    id_np = np.eye(P, dtype=np.float32).astype(NPBF16)
    ones64_np = np.ones((1, 64), dtype=np.float32).astype(NPBF16)
    ones512_np = np.ones((1, 512), dtype=np.float32).astype(NPBF16)

    in_maps = []
    for c in range(NCORES):
        lo, hi = c * HPC * D, (c + 1) * HPC * D        # 128-wide head slice
        wq = w[:, lo:hi] * scale
        wk = w[:, C + lo:C + hi]
        wv = w[:, 2 * C + lo:2 * C + hi]
        wqkv_c = np.concatenate([wq, wk, wv], axis=1).astype(NPBF16)
        wp_c = np.ascontiguousarray(wpj[lo:hi, :]).astype(NPBF16)
        m = {
            "xt": xf,
            "wqkv": wqkv_c,
            "wp": wp_c,
            "tri": tri_np,
            "ident": id_np,
            "ones64": ones64_np,
        }
        if with_bias:
            bq = ba[lo:hi] * scale
            bk = ba[C + lo:C + hi]
            bv = ba[2 * C + lo:2 * C + hi]
            m["bqkv"] = np.concatenate([bq, bk, bv])[None, :].astype(NPBF16)
            m["ones512"] = ones512_np
        in_maps.append(m)
    return in_maps, with_bias


def _combine(results, b_proj):
    acc = np.zeros((B, C, T), dtype=np.float32)
    for r in results:
        acc += np.asarray(r["outT"], dtype=np.float32)
    out = np.transpose(acc, (0, 2, 1))                 # [B, T, C]
    out = out + np.asarray(b_proj, dtype=np.float32)[None, None, :]
    return np.ascontiguousarray(out.astype(np.float32))


def run(x, w_attn, b_attn, w_proj, b_proj, trace=False, trace_cores=None):
    in_maps, with_bias = _prep_inputs(x, w_attn, b_attn, w_proj)
    nc = _get_nc(with_bias)
    res = run_bass_kernel_spmd(
        nc, in_maps, core_ids=list(range(NCORES)),
        trace=trace, trace_cores=trace_cores,
    )
    return _combine(res.results, b_proj), res


def kernel(x, w_attn, b_attn, w_proj, b_proj):
    out, _ = run(x, w_attn, b_attn, w_proj, b_proj, trace=False)
    return out
